# revision 5
# baseline (speedup 1.0000x reference)
"""Trainium2 Bass kernel for nn_LocalEncoder, v4.

v3 plus time-splitting of the long chunk: chunk1 (the 256 longest rows per
core) is scanned by TWO concurrent chains — chain1 covers t in [0, Ls), chain2
covers t in [Ls, T) after a 16-step warm-up from h=0 (the GRU's update gate
contracts ~0.5x/step, so the initial state is forgotten to ~1e-5 within the
warm-up). Rows with len <= Ls are fed an all-masked mask in chain2, so their
chain2 state stays exactly 0 and contributes nothing; their true last state
comes from chain1 via an on-device select. Three near-equal chains scan
concurrently (no solo tail); attention runs as a post-scan W=4 pipelined phase.
Host correction uses a per-row device step count.
"""
import sys
sys.path.insert(0, "/opt/trn_rl_repo")
from contextlib import ExitStack

import numpy as np
import ml_dtypes

import concourse.bass as bass
import concourse.bacc as bacc
import concourse.tile as tile
from concourse import mybir
from concourse import bass_utils

bf16 = ml_dtypes.bfloat16
AF = mybir.ActivationFunctionType
OP = mybir.AluOpType

B, T, E, U = 4096, 200, 100, 100
NCORES = 8
BC = 256
NCHUNK = 2
PERCORE = BC * NCHUNK
WX = 8
WA = 4
UP = U + 1

_CACHE = {}


def _plan(L0, L1):
    """Chain plan: (chunk, t0, t1, warm). Chain2 warm-starts WU before Ls."""
    WU = 16 if L1 >= 64 else 8
    rup = lambda a: ((a + WX - 1) // WX) * WX
    Ls = min(L1, rup((L1 + WU) // 2))
    if Ls >= L1:           # degenerate: no split
        return [(0, 0, L0, 0), (1, 0, L1, 0)], L1
    return [(0, 0, L0, 0), (1, 0, Ls, 0), (1, Ls - WU, L1, WU)], Ls


def _build(L0, L1):
    chains, Ls = _plan(L0, L1)
    NCH = len(chains)
    nblk = max((t1 - t0) // WX for _, t0, t1, _ in chains)
    nc = bacc.Bacc()
    dt = mybir.dt
    xaug = nc.dram_tensor("xaug", [NCH, nblk, 128, WX, BC], dt.bfloat16,
                          kind="ExternalInput")
    wmsel = nc.dram_tensor("wmsel", [128, BC], dt.bfloat16, kind="ExternalInput")
    wKz = nc.dram_tensor("wKz", [128, UP], dt.bfloat16, kind="ExternalInput")
    wKr = nc.dram_tensor("wKr", [128, UP], dt.bfloat16, kind="ExternalInput")
    wKh = nc.dram_tensor("wKh", [128, UP], dt.bfloat16, kind="ExternalInput")
    wRz = nc.dram_tensor("wRz", [UP, UP], dt.bfloat16, kind="ExternalInput")
    wRr = nc.dram_tensor("wRr", [UP, UP], dt.bfloat16, kind="ExternalInput")
    wRh = nc.dram_tensor("wRh", [UP, UP], dt.bfloat16, kind="ExternalInput")
    wA1 = nc.dram_tensor("wA1", [U, U], dt.bfloat16, kind="ExternalInput")
    wA2 = nc.dram_tensor("wA2", [U, U], dt.bfloat16, kind="ExternalInput")
    wVr = nc.dram_tensor("wVr", [U, U], dt.bfloat16, kind="ExternalInput")
    outraw = nc.dram_tensor("outraw", [NCHUNK, U, BC], dt.float32, kind="ExternalOutput")
    lastout = nc.dram_tensor("lastout", [NCHUNK, U, BC], dt.float32, kind="ExternalOutput")

    with tile.TileContext(nc) as tc, ExitStack() as octx:
        singles = octx.enter_context(tc.tile_pool(name="singles", bufs=1))
        dram = octx.enter_context(tc.tile_pool(name="dram", bufs=1, space="DRAM"))

        def load_w(dram_w, p, m):
            t = singles.tile([p, m], mybir.dt.bfloat16, tag=dram_w.name)
            nc.sync.dma_start(out=t, in_=dram_w[:, :])
            return t
        Kz, Kr, Kh = load_w(wKz, 128, UP), load_w(wKr, 128, UP), load_w(wKh, 128, UP)
        Rz, Rr, Rh = load_w(wRz, UP, UP), load_w(wRr, UP, UP), load_w(wRh, UP, UP)
        A1b, A2b, Vr = load_w(wA1, U, U), load_w(wA2, U, U), load_w(wVr, U, U)
        msel = load_w(wmsel, 128, BC)

        state = dram.tile([NCHUNK, U, L1, BC], mybir.dt.bfloat16)
        hfin = [None] * NCH

        # ---------------- scan: all chains concurrently ----------------
        with ExitStack() as ctx:
            xp = ctx.enter_context(tc.tile_pool(name="xp", bufs=3))
            hp = ctx.enter_context(tc.tile_pool(name="hp", bufs=4))
            gp = ctx.enter_context(tc.tile_pool(name="gp", bufs=3))
            pzr = ctx.enter_context(tc.tile_pool(name="pzr", bufs=1, space="PSUM"))
            phc = ctx.enter_context(tc.tile_pool(name="phc", bufs=1, space="PSUM"))

            hprev = [None] * NCH
            for i in range(NCH):
                h0 = hp.tile([128, BC], mybir.dt.bfloat16, tag=f"h{i}", name=f"h{i}")
                nc.vector.memset(h0, 0.0)
                nc.vector.memset(h0[96:128, :], 1.0)
                nc.vector.memset(h0[96:100, :], 0.0)
                hprev[i] = h0

            xts = [None] * NCH
            rounds = max(t1 - t0 for _, t0, t1, _ in chains)
            for r in range(rounds):
                alive = [i for i, (_, t0, t1, _) in enumerate(chains)
                         if r < t1 - t0]
                ib = r % WX
                if ib == 0:
                    for i in alive:
                        xt = xp.tile([128, WX, BC], mybir.dt.bfloat16, tag=f"x{i}",
                                     name=f"x{i}")
                        nc.sync.dma_start(out=xt, in_=xaug[i, r // WX, :, :, :])
                        xts[i] = xt
                zr = [None] * NCH
                hc = [None] * NCH
                for i in alive:
                    zr[i] = pzr.tile([128, 2, BC], mybir.dt.float32, tag=f"zr{i}",
                                     name=f"zr{i}")
                    hc[i] = phc.tile([128, 2, BC], mybir.dt.float32, tag=f"hc{i}",
                                     name=f"hc{i}")
                for i in alive:
                    nc.tensor.matmul(zr[i][0:UP, 0, :], lhsT=Kz, rhs=xts[i][:, ib, :],
                                     start=True, stop=False, skip_group_check=True)
                for i in alive:
                    nc.tensor.matmul(zr[i][0:UP, 0, :], lhsT=Rz, rhs=hprev[i][0:UP, :],
                                     start=False, stop=True, skip_group_check=True)
                for i in alive:
                    nc.tensor.matmul(zr[i][0:UP, 1, :], lhsT=Kr, rhs=xts[i][:, ib, :],
                                     start=True, stop=False, skip_group_check=True)
                for i in alive:
                    nc.tensor.matmul(zr[i][0:UP, 1, :], lhsT=Rr, rhs=hprev[i][0:UP, :],
                                     start=False, stop=True, skip_group_check=True)
                for i in alive:
                    nc.tensor.matmul(hc[i][0:UP, 0, :], lhsT=Kh, rhs=xts[i][:, ib, :],
                                     start=True, stop=True, skip_group_check=True)
                for i in alive:
                    nc.tensor.matmul(hc[i][0:UP, 1, :], lhsT=Rh, rhs=hprev[i][0:UP, :],
                                     start=True, stop=True, skip_group_check=True)
                for i in alive:
                    ch, t0, t1, warm = chains[i]
                    t = t0 + r
                    h = hprev[i]
                    zrs = gp.tile([UP, 2, BC], mybir.dt.bfloat16, tag=f"zrs{i}",
                                  name=f"zrs{i}")
                    nc.scalar.activation(zrs, zr[i][0:UP, :, :], AF.Sigmoid)
                    t1t = gp.tile([UP, BC], mybir.dt.bfloat16, tag=f"t1{i}",
                                  name=f"t1{i}")
                    nc.vector.tensor_tensor(t1t, zrs[:, 1, :], hc[i][0:UP, 1, :],
                                            OP.mult)
                    s = gp.tile([UP, BC], mybir.dt.bfloat16, tag=f"s{i}", name=f"s{i}")
                    nc.vector.tensor_tensor(s, hc[i][0:UP, 0, :], t1t, OP.add)
                    hh = gp.tile([UP, BC], mybir.dt.bfloat16, tag=f"hh{i}",
                                 name=f"hh{i}")
                    nc.scalar.activation(hh, s, AF.Tanh)
                    d = gp.tile([UP, BC], mybir.dt.bfloat16, tag=f"d{i}", name=f"d{i}")
                    nc.vector.tensor_tensor(d, hh, h[0:UP, :], OP.subtract)
                    e = gp.tile([UP, BC], mybir.dt.bfloat16, tag=f"e{i}", name=f"e{i}")
                    nc.vector.tensor_tensor(e, zrs[:, 0, :], d, OP.mult)
                    hn = hp.tile([128, BC], mybir.dt.bfloat16, tag=f"h{i}",
                                 name=f"hn{i}")
                    nc.vector.tensor_tensor(hn[0:UP, :], e, h[0:UP, :], OP.add)
                    if r >= warm:
                        nc.sync.dma_start(out=state[ch, :, t, :], in_=hn[0:U, :])
                    hprev[i] = hn

            for i in range(NCH):
                hf = singles.tile([128, BC], mybir.dt.bfloat16, tag=f"hf{i}",
                                  name=f"hf{i}")
                nc.vector.tensor_copy(hf[0:UP, :], hprev[i][0:UP, :])
                hfin[i] = hf

        # ---------------- attention ----------------
        NG = [L0 // WA, L1 // WA]
        with ExitStack() as ctx:
            sp = ctx.enter_context(tc.tile_pool(name="sp", bufs=3))
            gp2 = ctx.enter_context(tc.tile_pool(name="gp2", bufs=3))
            ap2 = ctx.enter_context(tc.tile_pool(name="ap2", bufs=1))
            psb = ctx.enter_context(tc.tile_pool(name="psb", bufs=2, space="PSUM"))
            pal = ctx.enter_context(tc.tile_pool(name="pal", bufs=2, space="PSUM"))

            lasts, c4s, accs = [], [], []
            for c in range(NCHUNK):
                lt = ap2.tile([128, BC], mybir.dt.bfloat16, tag=f"lt{c}",
                              name=f"lt{c}")
                if c == 0 or len(chains) == 2:
                    nc.vector.tensor_copy(lt[0:UP, :], hfin[c][0:UP, :])
                else:
                    # lt = h1 + msel * (h2 - h1)
                    df = gp2.tile([UP, BC], mybir.dt.bfloat16, tag="df", name="df")
                    nc.vector.tensor_tensor(df, hfin[2][0:UP, :], hfin[1][0:UP, :],
                                            OP.subtract)
                    nc.vector.tensor_tensor(df, df, msel[0:UP, :], OP.mult)
                    nc.vector.tensor_tensor(lt[0:UP, :], hfin[1][0:UP, :], df,
                                            OP.add)
                lasts.append(lt)
                lo = gp2.tile([U, BC], mybir.dt.float32, tag="lo", name="lo")
                nc.vector.tensor_copy(lo, lt[0:U, :])
                nc.sync.dma_start(out=lastout[c, :, :], in_=lo)
                sb1 = psb.tile([128, WA, BC], mybir.dt.float32, tag="sb", name="sb1")
                nc.tensor.matmul(sb1[0:U, 0, :], lhsT=A1b, rhs=lt[0:U, :],
                                 start=True, stop=True)
                c4 = ap2.tile([U, WA, BC], mybir.dt.bfloat16, tag=f"c4{c}",
                              name=f"c4{c}")
                nc.vector.tensor_copy(c4[:, 0, :], sb1[0:U, 0, :])
                nc.gpsimd.tensor_copy(c4[:, 1, :], c4[:, 0, :])
                nc.gpsimd.tensor_copy(c4[:, 2:4, :], c4[:, 0:2, :])
                c4s.append(c4)
                acc = ap2.tile([U, WA, BC], mybir.dt.float32, tag=f"acc{c}",
                               name=f"acc{c}")
                nc.vector.memset(acc, 0.0)
                accs.append(acc)

            order = [(c, g) for g in range(max(NG)) for c in range(NCHUNK)
                     if g < NG[c]]
            for c, g in order:
                st4 = sp.tile([U, WA, BC], mybir.dt.bfloat16, tag="st", name="st4")
                nc.sync.dma_start(out=st4, in_=state[c, :, g * WA:(g + 1) * WA, :])
                sb4 = psb.tile([128, WA, BC], mybir.dt.float32, tag="sb", name="sb4")
                nc.tensor.matmul(sb4[0:U, 0:2, :], lhsT=A2b, rhs=st4[:, 0:2, :],
                                 start=True, stop=True)
                nc.tensor.matmul(sb4[0:U, 2:4, :], lhsT=A2b, rhs=st4[:, 2:4, :],
                                 start=True, stop=True)
                sbc = gp2.tile([U, WA, BC], mybir.dt.bfloat16, tag="sbc", name="sbc")
                nc.vector.tensor_tensor(sbc, sb4[0:U, :, :], c4s[c], OP.add)
                g4 = gp2.tile([U, WA, BC], mybir.dt.bfloat16, tag="g4", name="g4")
                nc.scalar.activation(g4, sbc, AF.Sigmoid)
                al4 = pal.tile([128, WA, BC], mybir.dt.float32, tag="al", name="al4")
                nc.tensor.matmul(al4[0:U, 0:2, :], lhsT=Vr, rhs=g4[:, 0:2, :],
                                 start=True, stop=True)
                nc.tensor.matmul(al4[0:U, 2:4, :], lhsT=Vr, rhs=g4[:, 2:4, :],
                                 start=True, stop=True)
                tmp = gp2.tile([U, WA, BC], mybir.dt.bfloat16, tag="tmp", name="tmp")
                nc.vector.tensor_tensor(tmp, al4[0:U, :, :], st4, OP.mult)
                if c == 0:
                    nc.gpsimd.tensor_tensor(accs[c], accs[c], tmp, OP.add)
                else:
                    nc.vector.tensor_tensor(accs[c], accs[c], tmp, OP.add)

            for c in range(NCHUNK):
                osb = gp2.tile([U, BC], mybir.dt.float32, tag=f"osb{c}",
                               name=f"osb{c}")
                nc.vector.tensor_reduce(
                    osb, accs[c].rearrange("u w b -> u b w"), mybir.AxisListType.X,
                    OP.add)
                nc.sync.dma_start(out=outraw[c, :, :], in_=osb)

    nc.compile()
    return nc, chains, Ls


def _prep_weights(kernel_w, rec_kernel, bias, A1_w, A2_w, v):
    b0, b1 = bias[0], bias[1]
    w = {}
    Kz = np.zeros((128, UP), np.float32)
    Kz[:E, :U] = -kernel_w[:, :U]
    Kz[100, :U] = -40.0
    Kz[101, :U] = -(b0[:U] + b1[:U])
    Kz[101, 100] = -40.0
    Kr = np.zeros((128, UP), np.float32)
    Kr[:E, :U] = kernel_w[:, U:2 * U]
    Kr[101, :U] = b0[U:2 * U] + b1[U:2 * U]
    Kh = np.zeros((128, UP), np.float32)
    Kh[:E, :U] = kernel_w[:, 2 * U:]
    Kh[101, :U] = b0[2 * U:]
    Rz = np.zeros((UP, UP), np.float32)
    Rz[:U, :U] = -rec_kernel[:, :U]
    Rr = np.zeros((UP, UP), np.float32)
    Rr[:U, :U] = rec_kernel[:, U:2 * U]
    Rh = np.zeros((UP, UP), np.float32)
    Rh[:U, :U] = rec_kernel[:, 2 * U:]
    Rh[100, :U] = b1[2 * U:]
    w["wKz"], w["wKr"], w["wKh"] = Kz, Kr, Kh
    w["wRz"], w["wRr"], w["wRh"] = Rz, Rr, Rh
    w["wA1"] = A1_w
    w["wA2"] = A2_w
    w["wVr"] = np.broadcast_to(v[0][:, None], (U, U)).copy()
    return {k: val.astype(bf16) for k, val in w.items()}


def _schedule(mask):
    lengths = mask.sum(1).astype(np.int64)
    order = np.argsort(lengths, kind="stable")
    Ls_ = lengths[order]
    L0 = int(Ls_[BC * NCORES - 1])
    L1 = int(Ls_[-1])
    rup = lambda a: min(T, ((a + WX - 1) // WX) * WX)
    return order, max(WX, rup(L0)), max(WX, rup(L1))


def _make_inmaps(session_hidden, mask, w, order, L0, L1, chains):
    nblk = max((t1 - t0) // WX for _, t0, t1, _ in chains)
    xs = session_hidden[order].reshape(PERCORE, NCORES, T, E)
    ms = mask[order].reshape(PERCORE, NCORES, T)
    Ls = chains[1][2] if len(chains) > 2 else L1
    in_maps = []
    for k in range(NCORES):
        xk = xs[:, k].reshape(NCHUNK, BC, T, E)
        mk = ms[:, k].reshape(NCHUNK, BC, T)
        lens_k = mk.sum(2)                       # [c, j]
        xa = np.zeros((len(chains), nblk, 128, WX, BC), np.float32)
        for i, (c, t0, t1, warm) in enumerate(chains):
            nb = (t1 - t0) // WX
            xseg = xk[c, :, t0:t1, :].transpose(2, 1, 0)    # [e, t, j] -> wait
            # xk[c] is [j, t, e]; want [e, tseg, j]
            xseg = xk[c, :, t0:t1, :].transpose(2, 1, 0)    # [e, tseg, j]
            xa[i, :nb, :E] = xseg.reshape(E, nb, WX, BC).transpose(1, 0, 2, 3)
            mseg = 1.0 - mk[c, :, t0:t1].T                  # [tseg, j]
            if warm > 0:
                # rows fully handled by the earlier chain: force all-masked
                dead = lens_k[c] <= Ls
                mseg = mseg.copy()
                mseg[:, dead] = 1.0
            xa[i, :nb, 100] = mseg.reshape(nb, WX, BC)
            xa[i, :nb, 101] = 1.0
        im = dict(w)
        im["xaug"] = xa.astype(bf16)
        msel = np.zeros((128, BC), np.float32)
        msel[:, :] = (lens_k[1] > Ls).astype(np.float32)[None, :]
        im["wmsel"] = msel.astype(bf16)
        in_maps.append(im)
    return in_maps


def kernel(session_hidden, mask, kernel, rec_kernel, bias, A1_w, A2_w, v):
    session_hidden = np.asarray(session_hidden, np.float32)
    mask = np.asarray(mask, np.float32)
    kernel_w = np.asarray(kernel, np.float32)
    rec_kernel = np.asarray(rec_kernel, np.float32)
    bias = np.asarray(bias, np.float32)
    A1_w = np.asarray(A1_w, np.float32)
    A2_w = np.asarray(A2_w, np.float32)
    v = np.asarray(v, np.float32)

    order, L0, L1 = _schedule(mask)
    key = (L0, L1)
    if _CACHE.get("key") != key:
        _CACHE["nc"], _CACHE["chains"], _CACHE["Ls"] = _build(L0, L1)
        _CACHE["key"] = key
    nc, chains, Ls = _CACHE["nc"], _CACHE["chains"], _CACHE["Ls"]

    w = _prep_weights(kernel_w, rec_kernel, bias, A1_w, A2_w, v)
    in_maps = _make_inmaps(session_hidden, mask, w, order, L0, L1, chains)
    res = bass_utils.run_bass_kernel_spmd(nc, in_maps, core_ids=list(range(NCORES)))

    out_s = np.zeros((B, U), np.float32)
    last_s = np.zeros((B, U), np.float32)
    tcs_row = np.zeros(B, np.float32)
    lengths_s = mask.sum(1)[order]
    for k in range(NCORES):
        r = res.results[k]
        for c in range(NCHUNK):
            ranks = (np.arange(BC) + c * BC) * NCORES + k
            out_s[ranks] = np.asarray(r["outraw"][c]).T.astype(np.float32)
            last_s[ranks] = np.asarray(r["lastout"][c]).T.astype(np.float32)
            if c == 0:
                tcs_row[ranks] = L0
            elif len(chains) > 2:
                ln = lengths_s[ranks]
                tcs_row[ranks] = np.where(ln <= Ls, float(Ls), float(L1))
            else:
                tcs_row[ranks] = L1

    sl_ = last_s @ A2_w
    c_ = last_s @ A1_w
    sig = lambda a: 1.0 / (1.0 + np.exp(-a))
    a_dev = sig(sl_ + c_) @ v[0]
    a_true = sig(sl_) @ v[0]
    n_dev = tcs_row - lengths_s
    out_sorted = (out_s
                  - n_dev[:, None] * (a_dev - a_true)[:, None] * last_s
                  + (T - tcs_row)[:, None] * a_true[:, None] * last_s)
    out = np.zeros((B, U), np.float32)
    out[order] = out_sorted
    return out.astype(np.float32)
